# revision 1
# baseline (speedup 1.0000x reference)
"""Fused LSTM-cell kernel for 8x Trainium2 NeuronCores (Bass/Tile).

Strategy: data-parallel over the batch. Each of the 8 cores handles 512
batch rows and computes all gates over the full hidden dim:

    gates[b, g, h] = x[b,:] @ W[g, h, :] + h_prev[b,:] @ V[g, h, :] + bias[g, h]

The two GEMMs are fused into one K=4096 contraction by concatenating
A = [x | h_prev] and stacking Wf = [W^T; V^T] (shared by all cores).
The 8192 fused output columns are reordered into 16 slabs of 512 where a
slab holds all 4 gates for 128 hidden columns — so each PSUM tile can be
combined into h_next/c_next immediately. Weights stream slab-by-slab
(~146 GB/s demand, well under HBM bandwidth), so the PE never waits on a
front-loaded weight burst. Matmul inputs are cast to fp16 on the host
(same 1 cycle/row PE rate as bf16 but 10 mantissa bits; PSUM accumulation
stays fp32); gate math runs in fp32 on ACT/DVE.
"""

import sys
import numpy as np

for _p in ("/opt/trn_rl_repo", "/root/.axon_site/_ro/trn_rl_repo"):
    if _p not in sys.path:
        sys.path.insert(0, _p)

import ml_dtypes

B = 4096
I_DIM = 2048
H_DIM = 2048
G = 4
N_CORES = 8
BS = B // N_CORES              # 512 batch rows per core
MT = BS // 128                 # 4 m-tiles per core
K_TOT = I_DIM + H_DIM          # 4096 fused contraction
KT = K_TOT // 128              # 32 k-tiles
HB = 128                       # hidden columns per slab
S = H_DIM // HB                # 16 slabs
SLAB_N = G * HB                # 512 output columns per slab (PSUM bank)
W_DMA_CHUNK = 8                # k-tiles per weight DMA (8*512*2B*128 = 1MB)
MM_DTYPE = "fp16"              # "fp16" | "bf16": fp16 is same PE speed, 8x accuracy

_COMPILED = None
TRACE = False          # test harness sets True to capture an NTFF profile
LAST_EXEC_NS = None
LAST_RESULT = None


def _build_program():
    import concourse.mybir as mybir
    import concourse.tile as tile
    from concourse import bacc

    dt = mybir.dt
    mm_dt = dt.float16 if MM_DTYPE == "fp16" else dt.bfloat16
    nc = bacc.Bacc("TRN2", target_bir_lowering=False, debug=False,
                   num_devices=N_CORES)

    a_dram = nc.dram_tensor("a_t", [MT, 128, K_TOT], mm_dt,
                            kind="ExternalInput").ap()
    w_dram = nc.dram_tensor("w_sl", [S, 128, KT, SLAB_N], mm_dt,
                            kind="ExternalInput").ap()
    bias_dram = nc.dram_tensor("bias_sl", [S, 128, SLAB_N], dt.float32,
                               kind="ExternalInput").ap()
    cprev_dram = nc.dram_tensor("c_prev_s", [BS, H_DIM], dt.float32,
                                kind="ExternalInput").ap()
    h_out = nc.dram_tensor("h_out", [BS, H_DIM], dt.float32,
                           kind="ExternalOutput").ap()
    c_out = nc.dram_tensor("c_out", [BS, H_DIM], dt.float32,
                           kind="ExternalOutput").ap()

    SIG = mybir.ActivationFunctionType.Sigmoid
    TANH = mybir.ActivationFunctionType.Tanh

    with tile.TileContext(nc) as tc:
        with (
            tc.tile_pool(name="apool", bufs=1) as apool,
            tc.tile_pool(name="wpool", bufs=2) as wpool,
            tc.tile_pool(name="bpool", bufs=2) as bpool,
            tc.tile_pool(name="cppool", bufs=6) as cppool,
            tc.tile_pool(name="psum", bufs=6, space="PSUM") as pspool,
            tc.tile_pool(name="gpool", bufs=2) as gpool,
            tc.tile_pool(name="actpool", bufs=2) as actpool,
            tc.tile_pool(name="tpool", bufs=2) as tpool,
            tc.tile_pool(name="opool", bufs=4) as opool,
        ):
            # Activations resident in SBUF. Interleave the leading a/w DMAs in
            # small chunks so the very first matmuls (m=0, low kt) have their
            # inputs a few microseconds earlier.
            a_all = apool.tile([128, MT, K_TOT], mm_dt, tag="a_all")
            w_first = wpool.tile([128, KT, SLAB_N], mm_dt, tag="w_sb")
            # Pre-warm the PE while the first DMAs land: the HAM clock gate
            # holds the PE at 1.2 GHz until it has been busy ~3.4us, so idling
            # through the DMA head would make the first ~30 real matmuls run
            # at half clock. Throwaway matmuls on a zeroed tile flip it to
            # 2.4 GHz before the real work arrives.
            warm = tpool.tile([128, 128], mm_dt, tag="warm")
            nc.any.memset(warm[:], 0.0)
            ps_w = pspool.tile([128, SLAB_N], dt.float32, tag="ps")
            # Pre-warm MMs end roughly when the first real inputs land, so
            # the HAM busy window never opens between preamble and work.
            for _ in range(32):
                nc.tensor.matmul(ps_w[:, 0:128], warm[:], warm[:])

            bias_first = bpool.tile([128, SLAB_N], dt.float32, tag="bias_sb")
            big_cfg = MT == 4 and KT == 32
            if big_cfg:
                # Slab 0 runs all 4 m-tiles as interleaved accumulation
                # groups: each weight k-chunk feeds 4 matmuls, so HBM demand
                # is ~288 GB/s — below supply — and the PE streams densely.
                # The order below lands every transfer ahead of its first use
                # (a-quarters q>=1 aren't needed until kt 8/16/24).
                QC = K_TOT // 4
                for m in range(MT):
                    nc.sync.dma_start(a_all[:, m, 0:QC], a_dram[m][:, 0:QC])
                nc.sync.dma_start(w_first[:, 0:2, :], w_dram[0, :, 0:2, :])
                nc.sync.dma_start(bias_first[:], bias_dram[0])
                nc.sync.dma_start(w_first[:, 2:4, :], w_dram[0, :, 2:4, :])
                nc.sync.dma_start(w_first[:, 4:6, :], w_dram[0, :, 4:6, :])
                nc.sync.dma_start(w_first[:, 6:8, :], w_dram[0, :, 6:8, :])
                for m in range(MT):
                    nc.sync.dma_start(a_all[:, m, QC:2 * QC],
                                      a_dram[m][:, QC:2 * QC])
                for kc in range(8, 16, 2):
                    nc.sync.dma_start(w_first[:, kc:kc + 2, :],
                                      w_dram[0, :, kc:kc + 2, :])
                for m in range(MT):
                    nc.sync.dma_start(a_all[:, m, 2 * QC:3 * QC],
                                      a_dram[m][:, 2 * QC:3 * QC])
                nc.sync.dma_start(w_first[:, 16:24, :], w_dram[0, :, 16:24, :])
                for m in range(MT):
                    nc.sync.dma_start(a_all[:, m, 3 * QC:4 * QC],
                                      a_dram[m][:, 3 * QC:4 * QC])
                nc.sync.dma_start(w_first[:, 24:32, :], w_dram[0, :, 24:32, :])
            else:
                for m in range(MT):
                    nc.sync.dma_start(a_all[:, m, :], a_dram[m])
                for kc in range(0, KT, W_DMA_CHUNK):
                    kc2 = min(kc + W_DMA_CHUNK, KT)
                    nc.sync.dma_start(w_first[:, kc:kc2, :],
                                      w_dram[0, :, kc:kc2, :])
                nc.sync.dma_start(bias_first[:], bias_dram[0])

            for s in range(S):
                # Interleave only slab 0's first two m-tiles (supply-bound);
                # sequential groups elsewhere keep each epilogue overlapped
                # with the next group's matmuls — including the kernel tail.
                if s == 0 and big_cfg:
                    mpairs = [(0, 1, 2, 3)]
                else:
                    mpairs = [(m,) for m in range(MT)]
                if s == 0:
                    w_sb, bias_sb = w_first, bias_first
                else:
                    w_sb = wpool.tile([128, KT, SLAB_N], mm_dt, tag="w_sb")
                    for kc in range(0, KT, W_DMA_CHUNK):
                        nc.sync.dma_start(w_sb[:, kc:kc + W_DMA_CHUNK, :],
                                          w_dram[s, :, kc:kc + W_DMA_CHUNK, :])
                    bias_sb = bpool.tile([128, SLAB_N], dt.float32,
                                         tag="bias_sb")
                    nc.sync.dma_start(bias_sb[:], bias_dram[s])

                for pair in mpairs:
                    cps, pss = {}, {}
                    for m in pair:
                        cp_sb = cppool.tile([128, HB], dt.float32, tag="cp_sb")
                        nc.sync.dma_start(
                            cp_sb[:],
                            cprev_dram[m * 128:(m + 1) * 128,
                                       s * HB:(s + 1) * HB])
                        cps[m] = cp_sb
                        pss[m] = pspool.tile([128, SLAB_N], dt.float32,
                                             tag="ps", name=f"ps_{s}_{m}")
                    for kt in range(KT):
                        for m in pair:
                            nc.tensor.matmul(
                                pss[m][:],
                                a_all[:, m, kt * 128:(kt + 1) * 128],
                                w_sb[:, kt, :],
                                start=(kt == 0),
                                stop=(kt == KT - 1),
                            )
                    for m in pair:
                        # PSUM eviction fused with the per-column bias add
                        g_sb = gpool.tile([128, SLAB_N], dt.float32,
                                          tag="g_sb")
                        nc.vector.tensor_add(g_sb[:], pss[m][:], bias_sb[:])
                        acts = actpool.tile([128, SLAB_N], dt.float32,
                                            tag="acts")
                        nc.scalar.activation(acts[:, 0:3 * HB],
                                             g_sb[:, 0:3 * HB], SIG)
                        nc.scalar.activation(acts[:, 3 * HB:4 * HB],
                                             g_sb[:, 3 * HB:4 * HB], TANH)

                        t0 = tpool.tile([128, HB], dt.float32, tag="t0")
                        nc.vector.tensor_mul(t0[:], acts[:, 0:HB], cps[m][:])
                        t1 = tpool.tile([128, HB], dt.float32, tag="t1")
                        nc.vector.tensor_mul(t1[:], acts[:, HB:2 * HB],
                                             acts[:, 3 * HB:4 * HB])
                        c_t = opool.tile([128, HB], dt.float32, tag="c_t")
                        nc.vector.tensor_add(c_t[:], t0[:], t1[:])
                        th = tpool.tile([128, HB], dt.float32, tag="th")
                        nc.scalar.activation(th[:], c_t[:], TANH)
                        h_t = opool.tile([128, HB], dt.float32, tag="h_t")
                        nc.vector.tensor_mul(h_t[:], acts[:, 2 * HB:3 * HB],
                                             th[:])

                        nc.sync.dma_start(
                            c_out[m * 128:(m + 1) * 128,
                                  s * HB:(s + 1) * HB], c_t[:])
                        nc.sync.dma_start(
                            h_out[m * 128:(m + 1) * 128,
                                  s * HB:(s + 1) * HB], h_t[:])

    nc.compile()
    return nc


def _prep_inputs(x, h_prev, c_prev, W, bW, V, bV, b):
    mm_np = np.float16 if MM_DTYPE == "fp16" else ml_dtypes.bfloat16
    x = np.asarray(x, np.float32)
    h_prev = np.asarray(h_prev, np.float32)
    c_prev = np.asarray(c_prev, np.float32)
    W = np.asarray(W, np.float32)
    bW = np.asarray(bW, np.float32)
    V = np.asarray(V, np.float32)
    bV = np.asarray(bV, np.float32)
    b = np.asarray(b, np.float32)

    A = np.concatenate([x, h_prev], axis=1).astype(mm_np)        # [B, K]

    # Fused weights, shared by all cores.
    # w_sl[s, p, kt, g*HB + jj] = WV[g, s*HB + jj, kt*128 + p]
    WV = np.concatenate([W, V], axis=2).astype(mm_np)            # [G, H, K]
    w_sl = np.ascontiguousarray(
        WV.reshape(G, S, HB, KT, 128).transpose(1, 4, 3, 0, 2)
    ).reshape(S, 128, KT, SLAB_N)

    bias_full = (bW + bV + b).astype(np.float32)                # [G, H]
    # bias_sl[s, p, g*HB + jj] = bias_full[g, s*HB + jj]
    bias_row = bias_full.reshape(G, S, HB).transpose(1, 0, 2).reshape(S, SLAB_N)
    bias_sl = np.ascontiguousarray(
        np.broadcast_to(bias_row[:, None, :], (S, 128, SLAB_N)))

    in_maps = []
    for c in range(N_CORES):
        r0, r1 = c * BS, (c + 1) * BS
        # a_t[m, p, kt*128 + j] = A[r0 + m*128 + j, kt*128 + p]
        a_t = np.ascontiguousarray(
            A[r0:r1].reshape(MT, 128, KT, 128).transpose(0, 3, 2, 1)
        ).reshape(MT, 128, K_TOT)
        in_maps.append({
            "a_t": a_t,
            "w_sl": w_sl,
            "bias_sl": bias_sl,
            "c_prev_s": np.ascontiguousarray(c_prev[r0:r1]),
        })
    return in_maps


def kernel(x, h_prev, c_prev, W, bW, V, bV, b):
    global _COMPILED
    from concourse.bass_utils import run_bass_kernel_spmd

    if _COMPILED is None:
        _COMPILED = _build_program()
    nc = _COMPILED

    in_maps = _prep_inputs(x, h_prev, c_prev, W, bW, V, bV, b)
    res = run_bass_kernel_spmd(nc, in_maps, list(range(N_CORES)), trace=TRACE)
    global LAST_EXEC_NS, LAST_RESULT
    LAST_EXEC_NS = res.exec_time_ns
    LAST_RESULT = res

    h_next = np.concatenate([res.results[c]["h_out"] for c in range(N_CORES)],
                            axis=0)
    c_next = np.concatenate([res.results[c]["c_out"] for c in range(N_CORES)],
                            axis=0)
    return (h_next, c_next)



# revision 3
# speedup vs baseline: 1.2889x; 1.2889x over previous
"""Fused LSTM-cell kernel for 8x Trainium2 NeuronCores (Bass/Tile).

Data-parallel over batch (512 rows/core), transposed GEMM orientation:
weights are the PE-stationary operand, the batch is the N=512 moving dim,
so PSUM tiles are [128 hidden, 512 batch] and every matmul keeps the full
512-wide moving dim.

Mixed precision per gate (gate order f, i, o, c):
  - f, i (sigmoid-damped paths) run in fp8 e4m3 with DoubleRow perf mode:
    one matmul contracts K=256 (two k-subtiles paired per PE cell) at
    ~2x the bf16 rate. Host pre-scales A by 16 and W by 256 (TRN e4m3
    clips at +-240); the 1/4096 descale folds into the PSUM-evicting
    activation.
  - o, c (error-critical: o multiplies tanh(c) directly; c feeds tanh
    with unit slope) run in fp16.
  Simulated end-to-end rel_l2 vs the fp32 reference: ~1.3e-2.

The first 3 hidden chunks run with 6 PSUM accumulation groups interleaved
by k-tile so the PE streams while the (large) A operand is still landing
from HBM; the remaining 13 chunks run tile-sequential with pool-based
weight prefetch. Epilogue (sigmoid/tanh on ACT, elementwise on DVE)
overlaps the next chunk's matmuls.
"""

import sys
import numpy as np

for _p in ("/opt/trn_rl_repo", "/root/.axon_site/_ro/trn_rl_repo"):
    if _p not in sys.path:
        sys.path.insert(0, _p)

import ml_dtypes

B = 4096
I_DIM = 2048
H_DIM = 2048
G = 4
N_CORES = 8
BS = B // N_CORES              # 512 batch rows per core
NB = BS                        # moving free dim (batch)
K_TOT = I_DIM + H_DIM          # 4096 fused contraction
KT = K_TOT // 128              # 32 k-tiles
KT2 = KT // 2                  # 16 DoubleRow k-pairs
HC = H_DIM // 128              # 16 hidden chunks
HB = 128                       # hidden cols per chunk (PSUM partitions)
SA = 16.0                      # fp8 scale on A
SW = 256.0                     # fp8 scale on W
DESCALE = 1.0 / (SA * SW)
N_START = 3                    # hidden chunks handled by the interleaved start

_COMPILED = None
TRACE = False          # test harness sets True to capture an NTFF profile
LAST_EXEC_NS = None
LAST_RESULT = None


def _build_program():
    import concourse.mybir as mybir
    import concourse.tile as tile
    from concourse import bacc

    dt = mybir.dt
    DR = mybir.MatmulPerfMode.DoubleRow
    SIG = mybir.ActivationFunctionType.Sigmoid
    TANH = mybir.ActivationFunctionType.Tanh

    nc = bacc.Bacc("TRN2", target_bir_lowering=False, debug=False,
                   num_devices=N_CORES)

    a16_d = nc.dram_tensor("a16", [128, KT, NB], dt.float16,
                           kind="ExternalInput").ap()
    a8_d = nc.dram_tensor("a8", [128, KT, NB], dt.float8e4,
                          kind="ExternalInput").ap()
    # w*[gi, hc, p, kt, h]; fp8 holds gates (f, i), fp16 holds (o, c)
    w16_d = nc.dram_tensor("w16", [2, HC, 128, KT, HB], dt.float16,
                           kind="ExternalInput").ap()
    w8_d = nc.dram_tensor("w8", [2, HC, 128, KT, HB], dt.float8e4,
                          kind="ExternalInput").ap()
    bias_d = nc.dram_tensor("bias_t", [128, G * HC], dt.float32,
                            kind="ExternalInput").ap()
    cprev_d = nc.dram_tensor("cprev_t", [HC, 128, NB], dt.float32,
                             kind="ExternalInput").ap()
    hout_d = nc.dram_tensor("hout_t", [HC, 128, NB], dt.float32,
                            kind="ExternalOutput").ap()
    cout_d = nc.dram_tensor("cout_t", [HC, 128, NB], dt.float32,
                            kind="ExternalOutput").ap()

    with tile.TileContext(nc) as tc:
        with (
            tc.tile_pool(name="apool", bufs=1) as apool,
            tc.tile_pool(name="w16pool", bufs=7) as w16pool,
            tc.tile_pool(name="w8pool", bufs=7) as w8pool,
            tc.tile_pool(name="bpool", bufs=1) as bpool,
            tc.tile_pool(name="cppool", bufs=4) as cppool,
            tc.tile_pool(name="psum", bufs=8, space="PSUM") as pspool,
            tc.tile_pool(name="actpool", bufs=16) as actpool,
            tc.tile_pool(name="tpool", bufs=3) as tpool,
            tc.tile_pool(name="opool", bufs=6) as opool,
        ):
            a16_sb = apool.tile([128, KT, NB], dt.float16, tag="a16")
            a8_sb = apool.tile([128, KT, NB], dt.float8e4, tag="a8")
            bias_sb = bpool.tile([128, G * HC], dt.float32, tag="bias")

            def bias_ap(g, hc):
                return bias_sb[:, g * HC + hc:g * HC + hc + 1]

            # Pre-warm the PE so the HAM clock gate reaches 2.4 GHz by the
            # time the first real matmuls run (~3.4us of PE activity).
            warm = tpool.tile([128, 128], dt.float16, tag="warm")
            nc.any.memset(warm[:], 0.0)
            ps_w = pspool.tile([128, NB], dt.float32, tag="ps")
            for _ in range(32):
                nc.tensor.matmul(ps_w[:, 0:128], warm[:], warm[:])

            # ── startup supply: bias, c_prev, then a8/w8 interleaved ──────
            nc.sync.dma_start(bias_sb[:], bias_d)
            cps = {}
            for hc in range(N_START):
                cp = cppool.tile([128, NB], dt.float32, tag="cp",
                                 name=f"cp_{hc}")
                nc.sync.dma_start(cp[:], cprev_d[hc])
                cps[hc] = cp

            start_w8 = {}
            for hc in range(N_START):
                for gi in range(2):
                    start_w8[(gi, hc)] = w8pool.tile(
                        [128, KT, HB], dt.float8e4, tag="w8",
                        name=f"w8s_{gi}_{hc}")
            for c in range(0, KT, 8):
                nc.sync.dma_start(a8_sb[:, c:c + 8, :], a8_d[:, c:c + 8, :])
                for hc in range(N_START):
                    for gi in range(2):
                        nc.sync.dma_start(
                            start_w8[(gi, hc)][:, c:c + 8, :],
                            w8_d[gi, hc, :, c:c + 8, :])

            # ── phase A: fp8 gates (f, i) for hc 0..2, 6 interleaved groups
            ps8 = {}
            for hc in range(N_START):
                for gi in range(2):
                    ps8[(gi, hc)] = pspool.tile(
                        [128, NB], dt.float32, tag="ps",
                        name=f"ps8_{gi}_{hc}")
            for t2 in range(KT2):
                for hc in range(N_START):
                    for gi in range(2):
                        nc.tensor.matmul(
                            ps8[(gi, hc)][:],
                            start_w8[(gi, hc)][:, 2 * t2:2 * t2 + 2, :],
                            a8_sb[:, 2 * t2:2 * t2 + 2, :],
                            start=(t2 == 0), stop=(t2 == KT2 - 1),
                            perf_mode=DR)
            acts = {}
            for hc in range(N_START):
                for gi in range(2):
                    av = actpool.tile([128, NB], dt.float16, tag="act",
                                      name=f"act8_{gi}_{hc}")
                    nc.scalar.activation(av[:], ps8[(gi, hc)][:], SIG,
                                         bias=bias_ap(gi, hc), scale=DESCALE)
                    acts[(gi, hc)] = av

            # ── phase B: fp16 gates (o, c) for hc 0..2 ────────────────────
            start_w16 = {}
            for hc in range(N_START):
                for gi in range(2):
                    start_w16[(gi, hc)] = w16pool.tile(
                        [128, KT, HB], dt.float16, tag="w16",
                        name=f"w16s_{gi}_{hc}")
            for c in range(0, KT, 8):
                nc.sync.dma_start(a16_sb[:, c:c + 8, :], a16_d[:, c:c + 8, :])
                for hc in range(N_START):
                    for gi in range(2):
                        nc.sync.dma_start(
                            start_w16[(gi, hc)][:, c:c + 8, :],
                            w16_d[gi, hc, :, c:c + 8, :])
            ps16 = {}
            for hc in range(N_START):
                for gi in range(2):
                    ps16[(gi, hc)] = pspool.tile(
                        [128, NB], dt.float32, tag="ps",
                        name=f"ps16_{gi}_{hc}")
            for kt in range(KT):
                for hc in range(N_START):
                    for gi in range(2):
                        nc.tensor.matmul(
                            ps16[(gi, hc)][:],
                            start_w16[(gi, hc)][:, kt, :],
                            a16_sb[:, kt, :],
                            start=(kt == 0), stop=(kt == KT - 1))

            def epilogue(hc, act_f, act_i, act_o, act_c, cp):
                t0 = tpool.tile([128, NB], dt.float32, tag="t0")
                nc.vector.tensor_mul(t0[:], act_f[:], cp[:])
                t1 = tpool.tile([128, NB], dt.float32, tag="t1")
                nc.vector.tensor_mul(t1[:], act_i[:], act_c[:])
                c_t = opool.tile([128, NB], dt.float32, tag="c_t")
                nc.vector.tensor_add(c_t[:], t0[:], t1[:])
                th = tpool.tile([128, NB], dt.float32, tag="th")
                nc.scalar.activation(th[:], c_t[:], TANH)
                h_t = opool.tile([128, NB], dt.float32, tag="h_t")
                nc.vector.tensor_mul(h_t[:], act_o[:], th[:])
                nc.sync.dma_start(cout_d[hc], c_t[:])
                nc.sync.dma_start(hout_d[hc], h_t[:])

            for hc in range(N_START):
                for gi, fn in ((0, SIG), (1, TANH)):
                    av = actpool.tile([128, NB], dt.float16, tag="act",
                                      name=f"act16_{gi}_{hc}")
                    nc.scalar.activation(av[:], ps16[(gi, hc)][:], fn,
                                         bias=bias_ap(2 + gi, hc), scale=1.0)
                    acts[(2 + gi, hc)] = av
                epilogue(hc, acts[(0, hc)], acts[(1, hc)],
                         acts[(2, hc)], acts[(3, hc)], cps[hc])

            # ── steady state: hc 3..15, tile-sequential ───────────────────
            for hc in range(N_START, HC):
                cp = cppool.tile([128, NB], dt.float32, tag="cp")
                nc.sync.dma_start(cp[:], cprev_d[hc])
                wtiles = {}
                for gi in range(2):
                    w8t = w8pool.tile([128, KT, HB], dt.float8e4, tag="w8",
                                      name=f"w8_{hc}_{gi}")
                    nc.sync.dma_start(w8t[:], w8_d[gi, hc])
                    wtiles[gi] = w8t
                for gi in range(2):
                    w16t = w16pool.tile([128, KT, HB], dt.float16, tag="w16",
                                        name=f"w16_{hc}_{gi}")
                    nc.sync.dma_start(w16t[:], w16_d[gi, hc])
                    wtiles[2 + gi] = w16t

                hacts = {}
                for gi in range(2):
                    ps = pspool.tile([128, NB], dt.float32, tag="ps",
                                     name=f"ps_{hc}_{gi}")
                    for t2 in range(KT2):
                        nc.tensor.matmul(
                            ps[:], wtiles[gi][:, 2 * t2:2 * t2 + 2, :],
                            a8_sb[:, 2 * t2:2 * t2 + 2, :],
                            start=(t2 == 0), stop=(t2 == KT2 - 1),
                            perf_mode=DR)
                    av = actpool.tile([128, NB], dt.float16, tag="act",
                                      name=f"sact8_{hc}_{gi}")
                    nc.scalar.activation(av[:], ps[:], SIG,
                                         bias=bias_ap(gi, hc), scale=DESCALE)
                    hacts[gi] = av
                for gi, fn in ((0, SIG), (1, TANH)):
                    ps = pspool.tile([128, NB], dt.float32, tag="ps",
                                     name=f"ps_{hc}_{2 + gi}")
                    for kt in range(KT):
                        nc.tensor.matmul(
                            ps[:], wtiles[2 + gi][:, kt, :],
                            a16_sb[:, kt, :],
                            start=(kt == 0), stop=(kt == KT - 1))
                    av = actpool.tile([128, NB], dt.float16, tag="act",
                                      name=f"sact16_{hc}_{gi}")
                    nc.scalar.activation(av[:], ps[:], fn,
                                         bias=bias_ap(2 + gi, hc), scale=1.0)
                    hacts[2 + gi] = av
                epilogue(hc, hacts[0], hacts[1], hacts[2], hacts[3], cp)

    nc.compile()
    return nc


def _q8(t, scale):
    return np.clip(t * scale, -240.0, 240.0).astype(ml_dtypes.float8_e4m3)


def _prep_inputs(x, h_prev, c_prev, W, bW, V, bV, b):
    x = np.asarray(x, np.float32)
    h_prev = np.asarray(h_prev, np.float32)
    c_prev = np.asarray(c_prev, np.float32)
    W = np.asarray(W, np.float32)
    bW = np.asarray(bW, np.float32)
    V = np.asarray(V, np.float32)
    bV = np.asarray(bV, np.float32)
    b = np.asarray(b, np.float32)

    A = np.concatenate([x, h_prev], axis=1)                      # [B, K]
    WV = np.concatenate([W, V], axis=2)                          # [G, H, K]
    bias_full = (bW + bV + b)                                    # [G, H]

    # w16[gi, hc, p, kt, h] = WV[2+gi, hc*HB + h, kt*128 + p]
    w16 = np.ascontiguousarray(
        WV[2:4].astype(np.float16)
        .reshape(2, HC, HB, KT, 128).transpose(0, 1, 4, 3, 2))
    w8 = np.ascontiguousarray(
        _q8(WV[0:2], SW)
        .reshape(2, HC, HB, KT, 128).transpose(0, 1, 4, 3, 2))
    # bias_t[p, g*HC + hc] = bias_full[g, hc*HB + p]   (HB == 128)
    bias_t = np.ascontiguousarray(
        bias_full.reshape(G, HC, HB).transpose(2, 0, 1).reshape(128, G * HC))

    in_maps = []
    for core in range(N_CORES):
        r0, r1 = core * BS, (core + 1) * BS
        As = A[r0:r1]                                            # [BS, K]
        # a*[p, kt, b] = As[b, kt*128 + p]
        a16 = np.ascontiguousarray(
            As.astype(np.float16).T.reshape(KT, 128, BS).transpose(1, 0, 2))
        a8 = np.ascontiguousarray(
            _q8(As, SA).T.reshape(KT, 128, BS).transpose(1, 0, 2))
        # cprev_t[hc, h, b] = c_prev[r0 + b, hc*128 + h]
        cprev_t = np.ascontiguousarray(
            c_prev[r0:r1].T.reshape(HC, 128, BS))
        in_maps.append({
            "a16": a16,
            "a8": a8,
            "w16": w16,
            "w8": w8,
            "bias_t": bias_t,
            "cprev_t": cprev_t,
        })
    return in_maps


def kernel(x, h_prev, c_prev, W, bW, V, bV, b):
    global _COMPILED
    from concourse.bass_utils import run_bass_kernel_spmd

    if _COMPILED is None:
        _COMPILED = _build_program()
    nc = _COMPILED

    in_maps = _prep_inputs(x, h_prev, c_prev, W, bW, V, bV, b)
    res = run_bass_kernel_spmd(nc, in_maps, list(range(N_CORES)), trace=TRACE)
    global LAST_EXEC_NS, LAST_RESULT
    LAST_EXEC_NS = res.exec_time_ns
    LAST_RESULT = res

    h_parts, c_parts = [], []
    for core in range(N_CORES):
        ht = res.results[core]["hout_t"]          # [HC, 128, NB]
        ct = res.results[core]["cout_t"]
        h_parts.append(ht.transpose(2, 0, 1).reshape(BS, H_DIM))
        c_parts.append(ct.transpose(2, 0, 1).reshape(BS, H_DIM))
    h_next = np.concatenate(h_parts, axis=0)
    c_next = np.concatenate(c_parts, axis=0)
    return (h_next, c_next)


# revision 5
# speedup vs baseline: 1.3592x; 1.0545x over previous
"""Fused LSTM-cell kernel for 8x Trainium2 NeuronCores (Bass/Tile).

Data-parallel over batch (512 rows/core), transposed GEMM orientation:
weights are the PE-stationary operand, the batch is the N=512 moving dim,
so PSUM tiles are [128 hidden, 512 batch] and every matmul keeps the full
512-wide moving dim.

Mixed precision per gate (gate order f, i, o, c):
  - f, i run fully in fp8 e4m3 with DoubleRow perf mode: one matmul
    contracts K=256 (two k-subtiles paired per PE cell) at ~2x the bf16
    rate. Host pre-scales A by 16 and W by 256 (TRN e4m3 clips at +-240);
    the 1/4096 descale folds into the PSUM-evicting activation.
  - o runs half-K in fp8, half in fp16. Its fp16-half weights are
    pre-scaled by 4096 on the host so both halves accumulate into one
    PSUM group at a uniform scale.
  - c (the error-critical tanh path) runs fully in fp16.
  Full-batch simulated rel_l2 vs the fp32 reference: h 1.68e-2, c 1.28e-2.

The first 3 hidden chunks run with 6 PSUM accumulation groups interleaved
by k-tile so the PE streams while the (large) A operand is still landing
from HBM; the remaining 13 chunks run tile-sequential with pool-based
weight prefetch. Per chunk the gates are computed f, i, c, o with the
epilogue split around o so only the final h=o*tanh(c) multiply trails the
last matmul. Sigmoid/tanh run on ACT, elementwise on DVE, overlapped with
the next chunk's matmuls.
"""

import sys
import numpy as np

for _p in ("/opt/trn_rl_repo", "/root/.axon_site/_ro/trn_rl_repo"):
    if _p not in sys.path:
        sys.path.insert(0, _p)

import ml_dtypes

B = 4096
I_DIM = 2048
H_DIM = 2048
G = 4
N_CORES = 8
BS = B // N_CORES              # 512 batch rows per core
NB = BS                        # moving free dim (batch)
K_TOT = I_DIM + H_DIM          # 4096 fused contraction
KT = K_TOT // 128              # 32 k-tiles
KT2 = KT // 2                  # 16 DoubleRow k-pairs
KO8 = 16                       # o-gate k-tiles done in fp8 (k-tiles 0..15)
HC = H_DIM // 128              # 16 hidden chunks
HB = 128                       # hidden cols per chunk (PSUM partitions)
SA = 16.0                      # fp8 scale on A
SW = 256.0                     # fp8 scale on W
DESCALE = 1.0 / (SA * SW)
N_START = 3                    # hidden chunks handled by the interleaved start

_COMPILED = None
TRACE = False          # test harness sets True to capture an NTFF profile
LAST_EXEC_NS = None
LAST_RESULT = None

# DMA chunking of the k-tile axis: fine-grained at the head so the very
# first matmuls have their operands as early as possible.
_CHUNKS = [(0, 2), (2, 4), (4, 8), (8, 16), (16, 24), (24, 32)]


def _build_program():
    import concourse.mybir as mybir
    import concourse.tile as tile
    from concourse import bacc

    dt = mybir.dt
    DR = mybir.MatmulPerfMode.DoubleRow
    SIG = mybir.ActivationFunctionType.Sigmoid
    TANH = mybir.ActivationFunctionType.Tanh

    nc = bacc.Bacc("TRN2", target_bir_lowering=False, debug=False,
                   num_devices=N_CORES)

    a16_d = nc.dram_tensor("a16", [128, KT, NB], dt.float16,
                           kind="ExternalInput").ap()
    a8_d = nc.dram_tensor("a8", [128, KT, NB], dt.float8e4,
                          kind="ExternalInput").ap()
    w8fi_d = nc.dram_tensor("w8fi", [2, HC, 128, KT, HB], dt.float8e4,
                            kind="ExternalInput").ap()
    w8o_d = nc.dram_tensor("w8o", [HC, 128, KO8, HB], dt.float8e4,
                           kind="ExternalInput").ap()
    w16o_d = nc.dram_tensor("w16o", [HC, 128, KT - KO8, HB], dt.float16,
                            kind="ExternalInput").ap()
    w16c_d = nc.dram_tensor("w16c", [HC, 128, KT, HB], dt.float16,
                            kind="ExternalInput").ap()
    bias_d = nc.dram_tensor("bias_t", [128, G * HC], dt.float32,
                            kind="ExternalInput").ap()
    cprev_d = nc.dram_tensor("cprev_t", [HC, 128, NB], dt.float32,
                             kind="ExternalInput").ap()
    hout_d = nc.dram_tensor("hout_t", [HC, 128, NB], dt.float32,
                            kind="ExternalOutput").ap()
    cout_d = nc.dram_tensor("cout_t", [HC, 128, NB], dt.float32,
                            kind="ExternalOutput").ap()

    with tile.TileContext(nc) as tc:
        with (
            tc.tile_pool(name="apool", bufs=1) as apool,
            tc.tile_pool(name="w16pool", bufs=5) as w16pool,
            tc.tile_pool(name="w8pool", bufs=7) as w8pool,
            tc.tile_pool(name="wopool", bufs=3) as wopool,
            tc.tile_pool(name="bpool", bufs=1) as bpool,
            tc.tile_pool(name="cppool", bufs=3) as cppool,
            tc.tile_pool(name="psum", bufs=8, space="PSUM") as pspool,
            tc.tile_pool(name="actpool", bufs=12) as actpool,
            tc.tile_pool(name="tpool", bufs=2) as tpool,
            tc.tile_pool(name="opool", bufs=3) as opool,
        ):
            a16_sb = apool.tile([128, KT, NB], dt.float16, tag="a16")
            a8_sb = apool.tile([128, KT, NB], dt.float8e4, tag="a8")
            bias_sb = bpool.tile([128, G * HC], dt.float32, tag="bias")

            def bias_ap(g, hc):
                return bias_sb[:, g * HC + hc:g * HC + hc + 1]

            # Pre-warm the PE so the HAM clock gate reaches 2.4 GHz by the
            # time the first real matmuls run (~3.4us of PE activity).
            warm = tpool.tile([128, 128], dt.float16, tag="warm")
            nc.any.memset(warm[:], 0.0)
            ps_w = pspool.tile([128, NB], dt.float32, tag="ps")
            for _ in range(32):
                nc.tensor.matmul(ps_w[:, 0:128], warm[:], warm[:])

            # ── startup supply: a8 + fp8 weights, finest chunks first ─────
            start_w8 = {}
            for hc in range(N_START):
                for gi in range(2):
                    start_w8[(gi, hc)] = w8pool.tile(
                        [128, KT, HB], dt.float8e4, tag="w8",
                        name=f"w8s_{gi}_{hc}")
            for c0, c1 in _CHUNKS:
                nc.sync.dma_start(a8_sb[:, c0:c1, :], a8_d[:, c0:c1, :])
                for hc in range(N_START):
                    for gi in range(2):
                        nc.sync.dma_start(
                            start_w8[(gi, hc)][:, c0:c1, :],
                            w8fi_d[gi, hc, :, c0:c1, :])
            nc.sync.dma_start(bias_sb[:], bias_d)
            cps = {}
            for hc in range(N_START):
                cp = cppool.tile([128, NB], dt.float32, tag="cp",
                                 name=f"cp_{hc}")
                nc.sync.dma_start(cp[:], cprev_d[hc])
                cps[hc] = cp

            # ── phase A: fp8 gates (f, i) for hc 0..2, 6 interleaved groups
            ps8 = {}
            for hc in range(N_START):
                for gi in range(2):
                    ps8[(gi, hc)] = pspool.tile(
                        [128, NB], dt.float32, tag="ps",
                        name=f"ps8_{gi}_{hc}")
            for t2 in range(KT2):
                for hc in range(N_START):
                    for gi in range(2):
                        nc.tensor.matmul(
                            ps8[(gi, hc)][:],
                            start_w8[(gi, hc)][:, 2 * t2:2 * t2 + 2, :],
                            a8_sb[:, 2 * t2:2 * t2 + 2, :],
                            start=(t2 == 0), stop=(t2 == KT2 - 1),
                            perf_mode=DR)
            acts = {}
            for hc in range(N_START):
                for gi in range(2):
                    av = actpool.tile([128, NB], dt.float16, tag="act",
                                      name=f"act8_{gi}_{hc}")
                    nc.scalar.activation(av[:], ps8[(gi, hc)][:], SIG,
                                         bias=bias_ap(gi, hc), scale=DESCALE)
                    acts[(gi, hc)] = av

            # ── phase B supply: a16, c-gate fp16 weights, o-gate weights ──
            start_wc, start_wo8, start_wo16 = {}, {}, {}
            for hc in range(N_START):
                start_wc[hc] = w16pool.tile([128, KT, HB], dt.float16,
                                            tag="w16", name=f"w16s_{hc}")
                start_wo8[hc] = wopool.tile([128, KO8, HB], dt.float8e4,
                                            tag="wo8", name=f"wo8s_{hc}")
                start_wo16[hc] = wopool.tile([128, KT - KO8, HB], dt.float16,
                                             tag="wo16", name=f"wo16s_{hc}")
            for c0, c1 in _CHUNKS:
                nc.sync.dma_start(a16_sb[:, c0:c1, :], a16_d[:, c0:c1, :])
                for hc in range(N_START):
                    nc.sync.dma_start(start_wc[hc][:, c0:c1, :],
                                      w16c_d[hc, :, c0:c1, :])
                if c1 <= KO8:
                    for hc in range(N_START):
                        nc.sync.dma_start(start_wo8[hc][:, c0:c1, :],
                                          w8o_d[hc, :, c0:c1, :])
                else:
                    for hc in range(N_START):
                        nc.sync.dma_start(
                            start_wo16[hc][:, c0 - KO8:c1 - KO8, :],
                            w16o_d[hc, :, c0 - KO8:c1 - KO8, :])

            # ── phase B: c (fp16 full K) + o (fp8 low half, fp16 high) ────
            psb = {}
            for hc in range(N_START):
                psb[("c", hc)] = pspool.tile([128, NB], dt.float32, tag="ps",
                                             name=f"psc_{hc}")
                psb[("o", hc)] = pspool.tile([128, NB], dt.float32, tag="ps",
                                             name=f"pso_{hc}")
            for kt in range(KT):
                for hc in range(N_START):
                    nc.tensor.matmul(
                        psb[("c", hc)][:], start_wc[hc][:, kt, :],
                        a16_sb[:, kt, :],
                        start=(kt == 0), stop=(kt == KT - 1))
                if kt < KO8 and kt % 2 == 0:
                    t2 = kt // 2
                    for hc in range(N_START):
                        nc.tensor.matmul(
                            psb[("o", hc)][:],
                            start_wo8[hc][:, 2 * t2:2 * t2 + 2, :],
                            a8_sb[:, 2 * t2:2 * t2 + 2, :],
                            start=(t2 == 0), stop=False,
                            perf_mode=DR)
                elif kt >= KO8:
                    for hc in range(N_START):
                        nc.tensor.matmul(
                            psb[("o", hc)][:],
                            start_wo16[hc][:, kt - KO8, :],
                            a16_sb[:, kt, :],
                            start=False, stop=(kt == KT - 1))

            def epi_stage1(hc, act_f, act_i, act_c, cp):
                t0 = tpool.tile([128, NB], dt.float32, tag="t0")
                nc.vector.tensor_mul(t0[:], act_f[:], cp[:])
                t1 = tpool.tile([128, NB], dt.float32, tag="t1")
                nc.vector.tensor_mul(t1[:], act_i[:], act_c[:])
                c_t = opool.tile([128, NB], dt.float32, tag="c_t")
                nc.vector.tensor_add(c_t[:], t0[:], t1[:])
                th = tpool.tile([128, NB], dt.float32, tag="th")
                nc.scalar.activation(th[:], c_t[:], TANH)
                nc.sync.dma_start(cout_d[hc], c_t[:])
                return th

            def epi_stage2(hc, act_o, th):
                h_t = opool.tile([128, NB], dt.float32, tag="h_t")
                nc.vector.tensor_mul(h_t[:], act_o[:], th[:])
                nc.sync.dma_start(hout_d[hc], h_t[:])

            for hc in range(N_START):
                ac = actpool.tile([128, NB], dt.float16, tag="act",
                                  name=f"actc_{hc}")
                nc.scalar.activation(ac[:], psb[("c", hc)][:], TANH,
                                     bias=bias_ap(3, hc), scale=1.0)
                th = epi_stage1(hc, acts[(0, hc)], acts[(1, hc)], ac, cps[hc])
                ao = actpool.tile([128, NB], dt.float16, tag="act",
                                  name=f"acto_{hc}")
                nc.scalar.activation(ao[:], psb[("o", hc)][:], SIG,
                                     bias=bias_ap(2, hc), scale=DESCALE)
                epi_stage2(hc, ao, th)

            # ── steady state: hc 3..15, tile-sequential, order f,i,c,o ────
            for hc in range(N_START, HC):
                cp = cppool.tile([128, NB], dt.float32, tag="cp",
                                 name=f"cp_{hc}")
                nc.sync.dma_start(cp[:], cprev_d[hc])
                wf = w8pool.tile([128, KT, HB], dt.float8e4, tag="w8",
                                 name=f"w8_{hc}_0")
                nc.sync.dma_start(wf[:], w8fi_d[0, hc])
                wi = w8pool.tile([128, KT, HB], dt.float8e4, tag="w8",
                                 name=f"w8_{hc}_1")
                nc.sync.dma_start(wi[:], w8fi_d[1, hc])
                wc = w16pool.tile([128, KT, HB], dt.float16, tag="w16",
                                  name=f"w16_{hc}")
                nc.sync.dma_start(wc[:], w16c_d[hc])
                wo8 = wopool.tile([128, KO8, HB], dt.float8e4, tag="wo8",
                                  name=f"wo8_{hc}")
                nc.sync.dma_start(wo8[:], w8o_d[hc])
                wo16 = wopool.tile([128, KT - KO8, HB], dt.float16,
                                   tag="wo16", name=f"wo16_{hc}")
                nc.sync.dma_start(wo16[:], w16o_d[hc])

                gacts = {}
                for gi, wt in ((0, wf), (1, wi)):
                    ps = pspool.tile([128, NB], dt.float32, tag="ps",
                                     name=f"ps_{hc}_{gi}")
                    for t2 in range(KT2):
                        nc.tensor.matmul(
                            ps[:], wt[:, 2 * t2:2 * t2 + 2, :],
                            a8_sb[:, 2 * t2:2 * t2 + 2, :],
                            start=(t2 == 0), stop=(t2 == KT2 - 1),
                            perf_mode=DR)
                    av = actpool.tile([128, NB], dt.float16, tag="act",
                                      name=f"sact_{hc}_{gi}")
                    nc.scalar.activation(av[:], ps[:], SIG,
                                         bias=bias_ap(gi, hc), scale=DESCALE)
                    gacts[gi] = av

                psc = pspool.tile([128, NB], dt.float32, tag="ps",
                                  name=f"ps_{hc}_c")
                for kt in range(KT):
                    nc.tensor.matmul(psc[:], wc[:, kt, :], a16_sb[:, kt, :],
                                     start=(kt == 0), stop=(kt == KT - 1))
                ac = actpool.tile([128, NB], dt.float16, tag="act",
                                  name=f"sact_{hc}_c")
                nc.scalar.activation(ac[:], psc[:], TANH,
                                     bias=bias_ap(3, hc), scale=1.0)
                th = epi_stage1(hc, gacts[0], gacts[1], ac, cp)

                pso = pspool.tile([128, NB], dt.float32, tag="ps",
                                  name=f"ps_{hc}_o")
                for t2 in range(KO8 // 2):
                    nc.tensor.matmul(
                        pso[:], wo8[:, 2 * t2:2 * t2 + 2, :],
                        a8_sb[:, 2 * t2:2 * t2 + 2, :],
                        start=(t2 == 0), stop=False, perf_mode=DR)
                for kt in range(KO8, KT):
                    nc.tensor.matmul(pso[:], wo16[:, kt - KO8, :],
                                     a16_sb[:, kt, :],
                                     start=False, stop=(kt == KT - 1))
                ao = actpool.tile([128, NB], dt.float16, tag="act",
                                  name=f"sact_{hc}_o")
                nc.scalar.activation(ao[:], pso[:], SIG,
                                     bias=bias_ap(2, hc), scale=DESCALE)
                epi_stage2(hc, ao, th)

    nc.compile()
    return nc


def _q8(t, scale):
    return np.clip(t * scale, -240.0, 240.0).astype(ml_dtypes.float8_e4m3)


def _prep_inputs(x, h_prev, c_prev, W, bW, V, bV, b):
    x = np.asarray(x, np.float32)
    h_prev = np.asarray(h_prev, np.float32)
    c_prev = np.asarray(c_prev, np.float32)
    W = np.asarray(W, np.float32)
    bW = np.asarray(bW, np.float32)
    V = np.asarray(V, np.float32)
    bV = np.asarray(bV, np.float32)
    b = np.asarray(b, np.float32)

    A = np.concatenate([x, h_prev], axis=1)                      # [B, K]
    WV = np.concatenate([W, V], axis=2)                          # [G, H, K]
    bias_full = (bW + bV + b)                                    # [G, H]

    def wt(arr, np_dt):
        # [*, H, K'] -> [*, HC, 128(p), KT', HB]
        ng, h, k = arr.shape
        ktp = k // 128
        return np.ascontiguousarray(
            arr.astype(np_dt)
            .reshape(ng, HC, HB, ktp, 128).transpose(0, 1, 4, 3, 2))

    w8fi = wt(_q8(WV[0:2], SW), ml_dtypes.float8_e4m3)
    w8o = wt(_q8(WV[2:3, :, :KO8 * 128], SW), ml_dtypes.float8_e4m3)[0]
    # o-gate fp16 half carries the 4096x scale so both halves share one
    # PSUM accumulation group (evicted with a single 1/4096 descale).
    w16o = wt(WV[2:3, :, KO8 * 128:] * (SA * SW), np.float16)[0]
    w16c = wt(WV[3:4], np.float16)[0]
    # bias_t[p, g*HC + hc] = bias_full[g, hc*HB + p]   (HB == 128)
    bias_t = np.ascontiguousarray(
        bias_full.reshape(G, HC, HB).transpose(2, 0, 1).reshape(128, G * HC))

    in_maps = []
    for core in range(N_CORES):
        r0, r1 = core * BS, (core + 1) * BS
        As = A[r0:r1]                                            # [BS, K]
        # a*[p, kt, b] = As[b, kt*128 + p]
        a16 = np.ascontiguousarray(
            As.astype(np.float16).T.reshape(KT, 128, BS).transpose(1, 0, 2))
        a8 = np.ascontiguousarray(
            _q8(As, SA).T.reshape(KT, 128, BS).transpose(1, 0, 2))
        # cprev_t[hc, h, b] = c_prev[r0 + b, hc*128 + h]
        cprev_t = np.ascontiguousarray(
            c_prev[r0:r1].T.reshape(HC, 128, BS))
        in_maps.append({
            "a16": a16,
            "a8": a8,
            "w8fi": w8fi,
            "w8o": w8o,
            "w16o": w16o,
            "w16c": w16c,
            "bias_t": bias_t,
            "cprev_t": cprev_t,
        })
    return in_maps


def kernel(x, h_prev, c_prev, W, bW, V, bV, b):
    global _COMPILED
    from concourse.bass_utils import run_bass_kernel_spmd

    if _COMPILED is None:
        _COMPILED = _build_program()
    nc = _COMPILED

    in_maps = _prep_inputs(x, h_prev, c_prev, W, bW, V, bV, b)
    res = run_bass_kernel_spmd(nc, in_maps, list(range(N_CORES)), trace=TRACE)
    global LAST_EXEC_NS, LAST_RESULT
    LAST_EXEC_NS = res.exec_time_ns
    LAST_RESULT = res

    h_parts, c_parts = [], []
    for core in range(N_CORES):
        ht = res.results[core]["hout_t"]          # [HC, 128, NB]
        ct = res.results[core]["cout_t"]
        h_parts.append(ht.transpose(2, 0, 1).reshape(BS, H_DIM))
        c_parts.append(ct.transpose(2, 0, 1).reshape(BS, H_DIM))
    h_next = np.concatenate(h_parts, axis=0)
    c_next = np.concatenate(c_parts, axis=0)
    return (h_next, c_next)


# revision 7
# speedup vs baseline: 1.4045x; 1.0334x over previous
"""Fused LSTM-cell kernel for 8x Trainium2 NeuronCores (Bass/Tile).

Data-parallel over batch (512 rows/core), transposed GEMM orientation:
weights are the PE-stationary operand, the batch is the N=512 moving dim,
so PSUM tiles are [128 hidden, 512 batch] and every matmul keeps the full
512-wide moving dim.

Mixed precision per gate (gate order f, i, o, c):
  - f, i run fully in fp8 e4m3 with DoubleRow perf mode: one matmul
    contracts K=256 (two k-subtiles paired per PE cell) at ~2x the bf16
    rate. Host pre-scales A by 16 and W by 256 (TRN e4m3 clips at +-240);
    the 1/4096 descale folds into the PSUM-evicting activation.
  - o runs half-K in fp8, half in fp16. Its fp16-half weights are
    pre-scaled by 4096 on the host so both halves accumulate into one
    PSUM group at a uniform scale.
  - c (the error-critical tanh path) runs fully in fp16.
  Full-batch simulated rel_l2 vs the fp32 reference: h 1.68e-2, c 1.28e-2.

The first 3 hidden chunks run with 6 PSUM accumulation groups interleaved
by k-tile so the PE streams while the (large) A operand is still landing
from HBM; the remaining 13 chunks run tile-sequential with pool-based
weight prefetch. Per chunk the gates are computed f, i, c, o with the
epilogue split around o so only the final h=o*tanh(c) multiply trails the
last matmul. Sigmoid/tanh run on ACT, elementwise on DVE, overlapped with
the next chunk's matmuls.
"""

import sys
import numpy as np

for _p in ("/opt/trn_rl_repo", "/root/.axon_site/_ro/trn_rl_repo"):
    if _p not in sys.path:
        sys.path.insert(0, _p)

import ml_dtypes

B = 4096
I_DIM = 2048
H_DIM = 2048
G = 4
N_CORES = 8
BS = B // N_CORES              # 512 batch rows per core
NB = BS                        # moving free dim (batch)
K_TOT = I_DIM + H_DIM          # 4096 fused contraction
KT = K_TOT // 128              # 32 k-tiles
KT2 = KT // 2                  # 16 DoubleRow k-pairs
KO8 = 16                       # o-gate k-tiles done in fp8 (k-tiles 0..15)
HC = H_DIM // 128              # 16 hidden chunks
HB = 128                       # hidden cols per chunk (PSUM partitions)
SA = 16.0                      # fp8 scale on A
SW = 256.0                     # fp8 scale on W
DESCALE = 1.0 / (SA * SW)
N_START = 3                    # hidden chunks handled by the interleaved start

_COMPILED = None
TRACE = False          # test harness sets True to capture an NTFF profile
LAST_EXEC_NS = None
LAST_RESULT = None

# DMA chunking of the k-tile axis: fine-grained at the head so the very
# first matmuls have their operands as early as possible.
_CHUNKS = [(0, 2), (2, 4), (4, 8), (8, 16), (16, 24), (24, 32)]


def _build_program():
    import concourse.mybir as mybir
    import concourse.tile as tile
    from concourse import bacc

    dt = mybir.dt
    DR = mybir.MatmulPerfMode.DoubleRow
    SIG = mybir.ActivationFunctionType.Sigmoid
    TANH = mybir.ActivationFunctionType.Tanh

    nc = bacc.Bacc("TRN2", target_bir_lowering=False, debug=False,
                   num_devices=N_CORES)

    a16_d = nc.dram_tensor("a16", [128, KT, NB], dt.float16,
                           kind="ExternalInput").ap()
    a8_d = nc.dram_tensor("a8", [128, KT, NB], dt.float8e4,
                          kind="ExternalInput").ap()
    w8fi_d = nc.dram_tensor("w8fi", [2, HC, 128, KT, HB], dt.float8e4,
                            kind="ExternalInput").ap()
    w8o_d = nc.dram_tensor("w8o", [HC, 128, KO8, HB], dt.float8e4,
                           kind="ExternalInput").ap()
    w16o_d = nc.dram_tensor("w16o", [HC, 128, KT - KO8, HB], dt.float16,
                            kind="ExternalInput").ap()
    w16c_d = nc.dram_tensor("w16c", [HC, 128, KT, HB], dt.float16,
                            kind="ExternalInput").ap()
    wpa_d = nc.dram_tensor("wpackA", [128, 2 * KT2 * 2 * N_START, HB],
                           dt.float8e4, kind="ExternalInput").ap()
    wpb8_d = nc.dram_tensor("wpackB8", [128, KO8 * N_START, HB],
                            dt.float8e4, kind="ExternalInput").ap()
    wpb16_d = nc.dram_tensor("wpackB16", [128, (KT + KT - KO8) * N_START, HB],
                             dt.float16, kind="ExternalInput").ap()
    bias_d = nc.dram_tensor("bias_t", [128, G * HC], dt.float32,
                            kind="ExternalInput").ap()
    cprev_d = nc.dram_tensor("cprev_t", [HC, 128, NB], dt.float32,
                             kind="ExternalInput").ap()
    hout_d = nc.dram_tensor("hout_t", [HC, 128, NB], dt.float32,
                            kind="ExternalOutput").ap()
    cout_d = nc.dram_tensor("cout_t", [HC, 128, NB], dt.float32,
                            kind="ExternalOutput").ap()

    with tile.TileContext(nc) as tc:
        with (
            tc.tile_pool(name="apool", bufs=1) as apool,
            tc.tile_pool(name="w16pool", bufs=3) as w16pool,
            tc.tile_pool(name="w8pool", bufs=4) as w8pool,
            tc.tile_pool(name="wopool", bufs=2) as wopool,
            tc.tile_pool(name="bpool", bufs=1) as bpool,
            tc.tile_pool(name="cppool", bufs=2) as cppool,
            tc.tile_pool(name="psum", bufs=8, space="PSUM") as pspool,
            tc.tile_pool(name="packpool", bufs=1) as packpool,
            tc.tile_pool(name="actpool", bufs=10) as actpool,
            tc.tile_pool(name="tpool", bufs=2) as tpool,
            tc.tile_pool(name="opool", bufs=2) as opool,
        ):
            a16_sb = apool.tile([128, KT, NB], dt.float16, tag="a16")
            a8_sb = apool.tile([128, KT, NB], dt.float8e4, tag="a8")
            bias_sb = bpool.tile([128, G * HC], dt.float32, tag="bias")

            def bias_ap(g, hc):
                return bias_sb[:, g * HC + hc:g * HC + hc + 1]

            # Pre-warm the PE so the HAM clock gate reaches 2.4 GHz by the
            # time the first real matmuls run (~3.4us of PE activity).
            warm = tpool.tile([128, 128], dt.float16, tag="warm")
            nc.any.memset(warm[:], 0.0)
            ps_w = pspool.tile([128, NB], dt.float32, tag="ps")
            for _ in range(32):
                nc.tensor.matmul(ps_w[:, 0:128], warm[:], warm[:])

            # ── startup supply: a8 + packed phase-A weights, interleaved ──
            # wpackA idx layout: pair-of-rows 2*(t2*6 + hc*2 + gi), matching
            # the phase-A consumption order exactly; large contiguous DMA
            # lines (4KB/partition) instead of 256B per-tile chunks.
            NPA = 2 * KT2 * 2 * N_START
            wpa_sb = packpool.tile([128, NPA, HB], dt.float8e4, tag="wpa")
            pa_chunks = [(0, 24), (24, 48), (48, 96), (96, 144), (144, 192)]
            a8_it = iter(_CHUNKS)
            for c0, c1 in pa_chunks:
                try:
                    k0, k1 = next(a8_it)
                    nc.sync.dma_start(a8_sb[:, k0:k1, :], a8_d[:, k0:k1, :])
                except StopIteration:
                    pass
                nc.sync.dma_start(wpa_sb[:, c0:c1, :], wpa_d[:, c0:c1, :])
            for k0, k1 in a8_it:
                nc.sync.dma_start(a8_sb[:, k0:k1, :], a8_d[:, k0:k1, :])
            nc.sync.dma_start(bias_sb[:], bias_d)
            cps = {}
            for hc in range(N_START):
                cp = cppool.tile([128, NB], dt.float32, tag="cp",
                                 name=f"cp_{hc}")
                nc.sync.dma_start(cp[:], cprev_d[hc])
                cps[hc] = cp

            # ── phase A: fp8 gates (f, i) for hc 0..2, 6 interleaved groups
            ps8 = {}
            for hc in range(N_START):
                for gi in range(2):
                    ps8[(gi, hc)] = pspool.tile(
                        [128, NB], dt.float32, tag="ps",
                        name=f"ps8_{gi}_{hc}")
            for t2 in range(KT2):
                for hc in range(N_START):
                    for gi in range(2):
                        idx = 2 * (t2 * 2 * N_START + hc * 2 + gi)
                        nc.tensor.matmul(
                            ps8[(gi, hc)][:],
                            wpa_sb[:, idx:idx + 2, :],
                            a8_sb[:, 2 * t2:2 * t2 + 2, :],
                            start=(t2 == 0), stop=(t2 == KT2 - 1),
                            perf_mode=DR)
            acts = {}
            for hc in range(N_START):
                for gi in range(2):
                    av = actpool.tile([128, NB], dt.float16, tag="act",
                                      name=f"act8_{gi}_{hc}")
                    nc.scalar.activation(av[:], ps8[(gi, hc)][:], SIG,
                                         bias=bias_ap(gi, hc), scale=DESCALE)
                    acts[(gi, hc)] = av

            # ── phase B supply: a16 + packed c/o weights ──────────────────
            # wpackB16 idx: kt<16 -> 3*kt + hc (c only); kt>=16 ->
            # 48 + 6*(kt-16) + {0..2: c hc, 3..5: o hc}. wpackB8 idx:
            # pair 2*(t2*3 + hc) for the o-gate fp8 half (k-tiles 0..15).
            NPB16 = (KT + KT - KO8) * N_START
            NPB8 = KO8 * N_START
            wpb16_sb = packpool.tile([128, NPB16, HB], dt.float16,
                                     tag="wpb16")
            wpb8_sb = packpool.tile([128, NPB8, HB], dt.float8e4, tag="wpb8")
            b16_chunks = [(0, 24), (24, 48), (48, 72), (72, 96),
                          (96, 120), (120, 144)]
            a16_it = iter(_CHUNKS)
            for ci, (c0, c1) in enumerate(b16_chunks):
                try:
                    k0, k1 = next(a16_it)
                    nc.sync.dma_start(a16_sb[:, k0:k1, :], a16_d[:, k0:k1, :])
                except StopIteration:
                    pass
                nc.sync.dma_start(wpb16_sb[:, c0:c1, :], wpb16_d[:, c0:c1, :])
                if ci == 3:
                    nc.sync.dma_start(wpb8_sb[:, 0:24, :], wpb8_d[:, 0:24, :])
                elif ci == 4:
                    nc.sync.dma_start(wpb8_sb[:, 24:48, :],
                                      wpb8_d[:, 24:48, :])
            for k0, k1 in a16_it:
                nc.sync.dma_start(a16_sb[:, k0:k1, :], a16_d[:, k0:k1, :])

            # ── phase B: c (fp16 full K) + o (fp8 low half, fp16 high) ────
            psb = {}
            for hc in range(N_START):
                psb[("c", hc)] = pspool.tile([128, NB], dt.float32, tag="ps",
                                             name=f"psc_{hc}")
                psb[("o", hc)] = pspool.tile([128, NB], dt.float32, tag="ps",
                                             name=f"pso_{hc}")
            # o-gate matmuls all sit in the kt>=16 half so their PSUM
            # banks only need to free up (phase-A evictions) by mid-phase.
            for kt in range(KT):
                for hc in range(N_START):
                    cidx = (3 * kt + hc if kt < KO8
                            else 3 * KO8 + 6 * (kt - KO8) + hc)
                    nc.tensor.matmul(
                        psb[("c", hc)][:], wpb16_sb[:, cidx, :],
                        a16_sb[:, kt, :],
                        start=(kt == 0), stop=(kt == KT - 1))
                if kt >= KO8:
                    if kt % 2 == 0:
                        t2 = (kt - KO8) // 2
                        for hc in range(N_START):
                            pidx = 2 * (t2 * N_START + hc)
                            nc.tensor.matmul(
                                psb[("o", hc)][:],
                                wpb8_sb[:, pidx:pidx + 2, :],
                                a8_sb[:, 2 * t2:2 * t2 + 2, :],
                                start=(t2 == 0), stop=False,
                                perf_mode=DR)
                    for hc in range(N_START):
                        oidx = 3 * KO8 + 6 * (kt - KO8) + 3 + hc
                        nc.tensor.matmul(
                            psb[("o", hc)][:],
                            wpb16_sb[:, oidx, :],
                            a16_sb[:, kt, :],
                            start=False, stop=(kt == KT - 1))

            def epi_stage1(hc, act_f, act_i, act_c, cp):
                t0 = tpool.tile([128, NB], dt.float16, tag="t0")
                nc.vector.tensor_mul(t0[:], act_f[:], cp[:])
                t1 = tpool.tile([128, NB], dt.float16, tag="t1")
                nc.vector.tensor_mul(t1[:], act_i[:], act_c[:])
                c_t = opool.tile([128, NB], dt.float32, tag="c_t")
                nc.vector.tensor_add(c_t[:], t0[:], t1[:])
                th = tpool.tile([128, NB], dt.float32, tag="th")
                nc.scalar.activation(th[:], c_t[:], TANH)
                nc.sync.dma_start(cout_d[hc], c_t[:])
                return th

            def epi_stage2(hc, act_o, th):
                h_t = opool.tile([128, NB], dt.float32, tag="h_t")
                nc.vector.tensor_mul(h_t[:], act_o[:], th[:])
                nc.sync.dma_start(hout_d[hc], h_t[:])

            for hc in range(N_START):
                ac = actpool.tile([128, NB], dt.float16, tag="act",
                                  name=f"actc_{hc}")
                nc.scalar.activation(ac[:], psb[("c", hc)][:], TANH,
                                     bias=bias_ap(3, hc), scale=1.0)
                th = epi_stage1(hc, acts[(0, hc)], acts[(1, hc)], ac, cps[hc])
                ao = actpool.tile([128, NB], dt.float16, tag="act",
                                  name=f"acto_{hc}")
                nc.scalar.activation(ao[:], psb[("o", hc)][:], SIG,
                                     bias=bias_ap(2, hc), scale=DESCALE)
                epi_stage2(hc, ao, th)

            # ── steady state: hc 3..15, tile-sequential, order f,i,c,o ────
            for hc in range(N_START, HC):
                cp = cppool.tile([128, NB], dt.float32, tag="cp",
                                 name=f"cp_{hc}")
                nc.sync.dma_start(cp[:], cprev_d[hc])
                wf = w8pool.tile([128, KT, HB], dt.float8e4, tag="w8",
                                 name=f"w8_{hc}_0")
                nc.sync.dma_start(wf[:], w8fi_d[0, hc])
                wi = w8pool.tile([128, KT, HB], dt.float8e4, tag="w8",
                                 name=f"w8_{hc}_1")
                nc.sync.dma_start(wi[:], w8fi_d[1, hc])
                wc = w16pool.tile([128, KT, HB], dt.float16, tag="w16",
                                  name=f"w16_{hc}")
                nc.sync.dma_start(wc[:], w16c_d[hc])
                wo8 = wopool.tile([128, KO8, HB], dt.float8e4, tag="wo8",
                                  name=f"wo8_{hc}")
                nc.sync.dma_start(wo8[:], w8o_d[hc])
                wo16 = wopool.tile([128, KT - KO8, HB], dt.float16,
                                   tag="wo16", name=f"wo16_{hc}")
                nc.sync.dma_start(wo16[:], w16o_d[hc])

                gacts = {}
                for gi, wt in ((0, wf), (1, wi)):
                    ps = pspool.tile([128, NB], dt.float32, tag="ps",
                                     name=f"ps_{hc}_{gi}")
                    for t2 in range(KT2):
                        nc.tensor.matmul(
                            ps[:], wt[:, 2 * t2:2 * t2 + 2, :],
                            a8_sb[:, 2 * t2:2 * t2 + 2, :],
                            start=(t2 == 0), stop=(t2 == KT2 - 1),
                            perf_mode=DR)
                    av = actpool.tile([128, NB], dt.float16, tag="act",
                                      name=f"sact_{hc}_{gi}")
                    nc.scalar.activation(av[:], ps[:], SIG,
                                         bias=bias_ap(gi, hc), scale=DESCALE)
                    gacts[gi] = av

                psc = pspool.tile([128, NB], dt.float32, tag="ps",
                                  name=f"ps_{hc}_c")
                for kt in range(KT):
                    nc.tensor.matmul(psc[:], wc[:, kt, :], a16_sb[:, kt, :],
                                     start=(kt == 0), stop=(kt == KT - 1))
                ac = actpool.tile([128, NB], dt.float16, tag="act",
                                  name=f"sact_{hc}_c")
                nc.scalar.activation(ac[:], psc[:], TANH,
                                     bias=bias_ap(3, hc), scale=1.0)
                th = epi_stage1(hc, gacts[0], gacts[1], ac, cp)

                pso = pspool.tile([128, NB], dt.float32, tag="ps",
                                  name=f"ps_{hc}_o")
                for t2 in range(KO8 // 2):
                    nc.tensor.matmul(
                        pso[:], wo8[:, 2 * t2:2 * t2 + 2, :],
                        a8_sb[:, 2 * t2:2 * t2 + 2, :],
                        start=(t2 == 0), stop=False, perf_mode=DR)
                for kt in range(KO8, KT):
                    nc.tensor.matmul(pso[:], wo16[:, kt - KO8, :],
                                     a16_sb[:, kt, :],
                                     start=False, stop=(kt == KT - 1))
                ao = actpool.tile([128, NB], dt.float16, tag="act",
                                  name=f"sact_{hc}_o")
                nc.scalar.activation(ao[:], pso[:], SIG,
                                     bias=bias_ap(2, hc), scale=DESCALE)
                epi_stage2(hc, ao, th)

    nc.compile()
    return nc


def _q8(t, scale):
    return np.clip(t * scale, -240.0, 240.0).astype(ml_dtypes.float8_e4m3)


def _prep_inputs(x, h_prev, c_prev, W, bW, V, bV, b):
    x = np.asarray(x, np.float32)
    h_prev = np.asarray(h_prev, np.float32)
    c_prev = np.asarray(c_prev, np.float32)
    W = np.asarray(W, np.float32)
    bW = np.asarray(bW, np.float32)
    V = np.asarray(V, np.float32)
    bV = np.asarray(bV, np.float32)
    b = np.asarray(b, np.float32)

    A = np.concatenate([x, h_prev], axis=1)                      # [B, K]
    WV = np.concatenate([W, V], axis=2)                          # [G, H, K]
    bias_full = (bW + bV + b)                                    # [G, H]

    def wt(arr, np_dt):
        # [*, H, K'] -> [*, HC, 128(p), KT', HB]
        ng, h, k = arr.shape
        ktp = k // 128
        return np.ascontiguousarray(
            arr.astype(np_dt)
            .reshape(ng, HC, HB, ktp, 128).transpose(0, 1, 4, 3, 2))

    w8fi = wt(_q8(WV[0:2], SW), ml_dtypes.float8_e4m3)
    w8o = wt(_q8(WV[2:3, :, :KO8 * 128], SW), ml_dtypes.float8_e4m3)[0]
    # o-gate fp16 half carries the 4096x scale so both halves share one
    # PSUM accumulation group (evicted with a single 1/4096 descale).
    w16o = wt(WV[2:3, :, KO8 * 128:] * (SA * SW), np.float16)[0]
    w16c = wt(WV[3:4], np.float16)[0]
    # Packed startup weights for hc 0..N_START-1, laid out in exact
    # phase-A / phase-B consumption order for large-line head DMAs.
    # wpackA[p, 2*(t2*6 + hc*2 + gi) + pr, h] = w8fi[gi, hc, p, 2*t2+pr, h]
    wpackA = np.ascontiguousarray(
        w8fi[:, :N_START].reshape(2, N_START, 128, KT2, 2, HB)
        .transpose(2, 3, 1, 0, 4, 5).reshape(128, 2 * KT2 * 2 * N_START, HB))
    # wpackB8[p, 2*(t2*3 + hc) + pr, h] = w8o[hc, p, 2*t2+pr, h]
    wpackB8 = np.ascontiguousarray(
        w8o[:N_START].reshape(N_START, 128, KO8 // 2, 2, HB)
        .transpose(1, 2, 0, 3, 4).reshape(128, KO8 * N_START, HB))
    # wpackB16: kt<16 -> idx 3*kt+hc (c gate); kt>=16 -> idx
    # 48 + 6*(kt-16) + {0..2: c, 3..5: o}
    p1 = w16c[:N_START, :, :KO8, :].transpose(1, 2, 0, 3)
    c2 = w16c[:N_START, :, KO8:, :].transpose(1, 2, 0, 3)
    o2 = w16o[:N_START].transpose(1, 2, 0, 3)
    p2 = np.concatenate([c2, o2], axis=2)
    wpackB16 = np.ascontiguousarray(np.concatenate(
        [p1.reshape(128, N_START * KO8, HB),
         p2.reshape(128, N_START * 2 * (KT - KO8), HB)], axis=1))
    # bias_t[p, g*HC + hc] = bias_full[g, hc*HB + p]   (HB == 128)
    bias_t = np.ascontiguousarray(
        bias_full.reshape(G, HC, HB).transpose(2, 0, 1).reshape(128, G * HC))

    in_maps = []
    for core in range(N_CORES):
        r0, r1 = core * BS, (core + 1) * BS
        As = A[r0:r1]                                            # [BS, K]
        # a*[p, kt, b] = As[b, kt*128 + p]
        a16 = np.ascontiguousarray(
            As.astype(np.float16).T.reshape(KT, 128, BS).transpose(1, 0, 2))
        a8 = np.ascontiguousarray(
            _q8(As, SA).T.reshape(KT, 128, BS).transpose(1, 0, 2))
        # cprev_t[hc, h, b] = c_prev[r0 + b, hc*128 + h]
        cprev_t = np.ascontiguousarray(
            c_prev[r0:r1].T.reshape(HC, 128, BS))
        in_maps.append({
            "a16": a16,
            "a8": a8,
            "w8fi": w8fi,
            "w8o": w8o,
            "w16o": w16o,
            "w16c": w16c,
            "wpackA": wpackA,
            "wpackB8": wpackB8,
            "wpackB16": wpackB16,
            "bias_t": bias_t,
            "cprev_t": cprev_t,
        })
    return in_maps


def kernel(x, h_prev, c_prev, W, bW, V, bV, b):
    global _COMPILED
    from concourse.bass_utils import run_bass_kernel_spmd

    if _COMPILED is None:
        _COMPILED = _build_program()
    nc = _COMPILED

    in_maps = _prep_inputs(x, h_prev, c_prev, W, bW, V, bV, b)
    res = run_bass_kernel_spmd(nc, in_maps, list(range(N_CORES)), trace=TRACE)
    global LAST_EXEC_NS, LAST_RESULT
    LAST_EXEC_NS = res.exec_time_ns
    LAST_RESULT = res

    h_parts, c_parts = [], []
    for core in range(N_CORES):
        ht = res.results[core]["hout_t"]          # [HC, 128, NB]
        ct = res.results[core]["cout_t"]
        h_parts.append(ht.transpose(2, 0, 1).reshape(BS, H_DIM))
        c_parts.append(ct.transpose(2, 0, 1).reshape(BS, H_DIM))
    h_next = np.concatenate(h_parts, axis=0)
    c_next = np.concatenate(c_parts, axis=0)
    return (h_next, c_next)
